# revision 27
# baseline (speedup 1.0000x reference)
"""Trainium2 Bass kernel for nn_ConceptIntergation (histogram_binning).

Reference computation:
    counts[b,s,n] = sum_k one_hot(concepts[b,s,k], 129)[..., n]  (n < 128; 128 = padding)
    out[b,s,n,d]  = counts[b,s,n] * emb_table[n,d]

Strategy (data-parallel over batch, 8 cores; transposed class-major layout):
  - Each core handles B_LOC=8 batches -> R=1600 (b,s) rows. The kernel is
    HBM-store bound (26.2 MB bf16 per core), and each of the 16 SDMA
    engines is port-limited to ~27 GB/s, so everything is organized to
    keep all 16 engines saturated from as early as possible.
  - Device layout puts the CONCEPT CLASS n on the partition axis:
      countsT[n, r] = sum_k (concepts[r,k] == n)
      out_d[n, r]   = emb[n, d] * countsT[n, r]
    With n on partitions, emb[:, d] is a per-partition scalar, so the big
    multiply runs as DVE tensor_scalar in the 4x perf mode (~630ns per
    [128,1600] bf16 slice); the Scalar engine (activation Copy with
    per-partition scale AP) computes 16 of the 64 d-slices concurrently.
    The histogram is 4 tensor_scalar is_equal ops (one per k, 4x mode)
    with in-place tensor_tensor accumulation, interleaved with the 4
    k-major index chunk loads so the chain finishes right after the last
    chunk lands.
  - Stores issue on both HWDGE rings (Sync ring for DVE groups, ACT ring
    for ScalarE groups) in multi-d-slice batches, with small first groups
    so the SDMA engines start early. Host transposes shards to [r, n, d]
    and upcasts bf16->f32 with an exact bit shift.
"""

import numpy as np
import ml_dtypes

import concourse.bass as bass
import concourse.mybir as mybir
from concourse import bacc
from concourse.tile import TileContext
from concourse.bass_utils import run_bass_kernel_spmd

B, S, K = 64, 200, 4
N, D = 128, 64
NCORES = 8
B_LOC = B // NCORES             # 8
R = B_LOC * S                   # 1600 (b,s) rows per core
P = 128
RK = K * R                      # 6400
OUTW = D * R                    # 102400

# d-slice store groups: DVE owns d 0..47 (Sync ring), ScalarE d 48..63
# (ACT ring). d 0..7 are computed per r-half on half-counts so the first
# stores issue while the second histogram half is still accumulating.
_HALF_D = [(0, 2), (2, 4), (4, 6), (6, 8)]
_DVE_D = [(8, 12), (12, 18), (18, 24), (24, 30), (30, 36), (36, 42), (42, 48)]
_SCE_D = [(48, 50), (50, 52), (52, 56), (56, 60), (60, 64)]
RH = R // 2                     # 800

BF16 = mybir.dt.bfloat16
F32 = mybir.dt.float32

_NC_CACHE = {}


def _build_nc():
    nc = bacc.Bacc()
    idxb = nc.declare_dram_parameter("idxb", [1, RK], BF16, isOutput=False)
    pe = nc.declare_dram_parameter("pe", [P, 1 + D], F32, isOutput=False)
    out = nc.declare_dram_parameter("out", [P, OUTW], BF16, isOutput=True)

    mult = mybir.AluOpType.mult
    add = mybir.AluOpType.add
    is_eq = mybir.AluOpType.is_equal

    with TileContext(nc) as tc:
        with (
            tc.tile_pool(name="const", bufs=1) as cpool,
            tc.tile_pool(name="vout", bufs=6) as vpool,
            tc.tile_pool(name="hout", bufs=8) as hpool,
            tc.tile_pool(name="sout", bufs=3) as spool,
        ):
            # single-partition index row (26 KB instead of a 1.6 MB
            # replicated load); the idle Tensor engine broadcasts it to all
            # 128 partitions via a K=1 ones-matmul into PSUM.
            idx1 = cpool.tile([1, RK], BF16)
            nc.scalar.dma_start(out=idx1, in_=idxb[0:1, :])
            pe_sb = cpool.tile([P, 1 + D], F32)
            nc.sync.dma_start(out=pe_sb, in_=pe[:, :])
            pcol = pe_sb[:, 0:1]
            ones = cpool.tile([1, P], BF16)
            nc.vector.memset(ones, 1.0)

            # warm the ScalarE activation table during the input DMAs
            warm = cpool.tile([P, 1], F32)
            nc.scalar.copy(out=warm, in_=pcol)

            # countsT[n, r] = sum_k (idx[r,k] == n), per r-half, straight
            # from the PSUM broadcast tiles (is_equal is exact on f32 ints)
            counts = cpool.tile([P, R], BF16)
            ck = cpool.tile([P, R], BF16)

            def emit_hist(h, ppool, ones_t, pcol_t):
                pts = []
                RQ = RH // 2  # 400
                for k in range(K):
                    # 2 full PSUM banks; each matmult writes one bank only
                    # (<=512 fp32 cols, and start=True resets whole banks)
                    pt = ppool.tile([P, 1024], F32, tag="pidx")
                    c0 = k * R + h * RH
                    for q in range(2):
                        nc.tensor.matmul(
                            out=pt[:, q * 512 : q * 512 + RQ],
                            lhsT=ones_t[:, :],
                            rhs=idx1[:, c0 + q * RQ : c0 + (q + 1) * RQ],
                            start=True, stop=True,
                        )
                    pts.append(pt)
                cs = counts[:, h * RH : (h + 1) * RH]
                ks = ck[:, h * RH : (h + 1) * RH]

                def ts_eq(dst, k):
                    # plain per-bank slices only (no strided PSUM views)
                    for q in range(2):
                        nc.vector.tensor_scalar(
                            out=dst[:, q * RQ : (q + 1) * RQ],
                            in0=pts[k][:, q * 512 : q * 512 + RQ],
                            scalar1=pcol_t, scalar2=None, op0=is_eq,
                        )

                ts_eq(cs, 0)
                ts_eq(ks, 1)
                nc.vector.tensor_tensor(out=cs, in0=cs, in1=ks, op=add)
                ts_eq(ks, 2)
                nc.vector.tensor_tensor(out=cs, in0=cs, in1=ks, op=add)
                ts_eq(ks, 3)
                nc.vector.tensor_tensor(out=cs, in0=cs, in1=ks, op=add)

            out3 = out.rearrange("p (d r) -> p d r", r=R)

            def emit_halfgroup(d0, d1, h):
                gd = d1 - d0
                ob = hpool.tile([P, gd * RH], BF16, tag="hob")
                for i in range(gd):
                    nc.vector.tensor_scalar(
                        out=ob[:, i * RH : (i + 1) * RH],
                        in0=counts[:, h * RH : (h + 1) * RH],
                        scalar1=pe_sb[:, 1 + (d0 + i) : 2 + (d0 + i)],
                        scalar2=None, op0=mult,
                    )
                nc.sync.dma_start(
                    out=out3[:, d0:d1, h * RH : (h + 1) * RH],
                    in_=ob.rearrange("p (d r) -> p d r", r=RH),
                )
                return ob

            def emit_vgroup(d0, d1):
                ob = vpool.tile([P, (d1 - d0) * R], BF16, tag="vob")
                for i in range(d1 - d0):
                    nc.vector.tensor_scalar(
                        out=ob[:, i * R : (i + 1) * R], in0=counts,
                        scalar1=pe_sb[:, 1 + (d0 + i) : 2 + (d0 + i)],
                        scalar2=None, op0=mult,
                    )
                nc.sync.dma_start(out=out[:, d0 * R : d1 * R], in_=ob)

            def emit_sgroup(d0, d1):
                ob = spool.tile([P, (d1 - d0) * R], BF16, tag="sob")
                for i in range(d1 - d0):
                    nc.scalar.mul(
                        out=ob[:, i * R : (i + 1) * R], in_=counts,
                        mul=pe_sb[:, 1 + (d0 + i) : 2 + (d0 + i)],
                    )
                nc.scalar.dma_start(out=out[:, d0 * R : d1 * R], in_=ob)

            bp = mybir.AluOpType.bypass
            with tc.psum_pool(name="pidx", bufs=4) as ppool:
                emit_hist(0, ppool, ones, pcol)
                last_ob = None
                for d0, d1 in _HALF_D:
                    last_ob = emit_halfgroup(d0, d1, 0)
                # RAW-only gates (bypass copies that read a gating region):
                # chain B's matmuls recycle chain A's PSUM slots, so their
                # ones operand is cloned with a read of chain A's final
                # compare output; chain B's DVE compares would otherwise be
                # hoisted ahead of the first half-group muls by the
                # scheduler, so their pcol operand is cloned with a read of
                # the last h0 tile.
                ones_b = cpool.tile([1, P], BF16)
                nc.vector.scalar_tensor_tensor(
                    out=ones_b, in0=ones, scalar=0.0, in1=ck[0:1, 0:P],
                    op0=bp, op1=bp,
                )
                pcol_b = cpool.tile([P, 1], F32)
                nc.vector.scalar_tensor_tensor(
                    out=pcol_b, in0=pcol, scalar=0.0, in1=last_ob[:, 0:1],
                    op0=bp, op1=bp,
                )
                emit_hist(1, ppool, ones_b, pcol_b)
            for d0, d1 in _SCE_D:
                emit_sgroup(d0, d1)
            for d0, d1 in _HALF_D:
                emit_halfgroup(d0, d1, 1)
            for d0, d1 in _DVE_D:
                emit_vgroup(d0, d1)

    nc.finalize()
    return nc


def _get_nc():
    if "nc" not in _NC_CACHE:
        _NC_CACHE["nc"] = _build_nc()
    return _NC_CACHE["nc"]


def _prepare_in_maps(concepts, emb_table):
    concepts = np.asarray(concepts)
    emb = np.asarray(emb_table, dtype=np.float32)

    # per-core k-major index rows (single partition; device broadcasts)
    conc = concepts.reshape(NCORES, R, K)
    idx_dev = np.ascontiguousarray(
        conc.transpose(0, 2, 1).reshape(NCORES, 1, RK).astype(ml_dtypes.bfloat16)
    )

    pe = np.empty((P, 1 + D), dtype=np.float32)
    pe[:, 0] = np.arange(P, dtype=np.float32)
    pe[:, 1:] = emb

    return [{"idxb": idx_dev[i], "pe": pe} for i in range(NCORES)]


def _run(concepts, emb_table, **spmd_kwargs):
    nc = _get_nc()
    in_maps = _prepare_in_maps(concepts, emb_table)
    res = run_bass_kernel_spmd(nc, in_maps, core_ids=list(range(NCORES)), **spmd_kwargs)
    # shards are [128(n), 64(d)*1600(r)] bf16; -> [r, n, d], upcast exactly
    u16 = np.stack(
        [np.asarray(res.results[i]["out"]).view(np.uint16) for i in range(NCORES)]
    ).reshape(NCORES, N, D, R)
    u16 = u16.transpose(0, 3, 1, 2)  # -> [core, r, n, d]
    f32 = (u16.astype(np.uint32) << 16).view(np.float32)
    out = f32.reshape(B, S, N, D)
    return out, res


def kernel(concepts, emb_table):
    out, _ = _run(concepts, emb_table)
    return out


# revision 29
# speedup vs baseline: 1.1325x; 1.1325x over previous
"""Trainium2 Bass kernel for nn_ConceptIntergation (histogram_binning).

Reference computation:
    counts[b,s,n] = sum_k one_hot(concepts[b,s,k], 129)[..., n]  (n < 128; 128 = padding)
    out[b,s,n,d]  = counts[b,s,n] * emb_table[n,d]

Strategy (data-parallel over batch, 8 cores; transposed class-major layout):
  - Each core handles B_LOC=8 batches -> R=1600 (b,s) rows. The kernel is
    HBM-store bound (26.2 MB bf16 per core), and each of the 16 SDMA
    engines is port-limited to ~27 GB/s, so everything is organized to
    keep all 16 engines saturated from as early as possible.
  - Device layout puts the CONCEPT CLASS n on the partition axis:
      countsT[n, r] = sum_k (concepts[r,k] == n)
      out_d[n, r]   = emb[n, d] * countsT[n, r]
    With n on partitions, emb[:, d] is a per-partition scalar, so the big
    multiply runs as DVE tensor_scalar in the 4x perf mode (~630ns per
    [128,1600] bf16 slice); the Scalar engine (activation Copy with
    per-partition scale AP) computes 16 of the 64 d-slices concurrently.
    The histogram is 4 tensor_scalar is_equal ops (one per k, 4x mode)
    with in-place tensor_tensor accumulation, interleaved with the 4
    k-major index chunk loads so the chain finishes right after the last
    chunk lands.
  - Stores issue on both HWDGE rings (Sync ring for DVE groups, ACT ring
    for ScalarE groups) in multi-d-slice batches, with small first groups
    so the SDMA engines start early. Host transposes shards to [r, n, d]
    and upcasts bf16->f32 with an exact bit shift.
"""

import numpy as np
import ml_dtypes

import concourse.bass as bass
import concourse.mybir as mybir
from concourse import bacc
from concourse.tile import TileContext
from concourse.bass_utils import run_bass_kernel_spmd

B, S, K = 64, 200, 4
N, D = 128, 64
NCORES = 8
B_LOC = B // NCORES             # 8
R = B_LOC * S                   # 1600 (b,s) rows per core
P = 128
RK = K * R                      # 6400
OUTW = D * R                    # 102400

# d-slice store groups: DVE owns d 0..47 (Sync ring), ScalarE d 48..63
# (ACT ring). d 0..7 are computed per r-half on half-counts so the first
# stores issue while the second histogram half is still accumulating.
_HALF_D = [(0, 2), (2, 4), (4, 6), (6, 8)]
_DVE_D = [(8, 16), (16, 24), (24, 32), (32, 40), (40, 48)]
_SCE_D = [(48, 56), (56, 64)]
RH = R // 2                     # 800

BF16 = mybir.dt.bfloat16
F32 = mybir.dt.float32

_NC_CACHE = {}


def _build_nc():
    nc = bacc.Bacc()
    idxb = nc.declare_dram_parameter("idxb", [1, RK], BF16, isOutput=False)
    pe = nc.declare_dram_parameter("pe", [P, 1 + D], F32, isOutput=False)
    out = nc.declare_dram_parameter("out", [P, OUTW], BF16, isOutput=True)

    mult = mybir.AluOpType.mult
    add = mybir.AluOpType.add
    is_eq = mybir.AluOpType.is_equal

    with TileContext(nc) as tc:
        with (
            tc.tile_pool(name="const", bufs=1) as cpool,
            tc.tile_pool(name="vout", bufs=4) as vpool,
            tc.tile_pool(name="hout", bufs=8) as hpool,
            tc.tile_pool(name="sout", bufs=2) as spool,
        ):
            # single-partition index row (26 KB instead of a 1.6 MB
            # replicated load); the idle Tensor engine broadcasts it to all
            # 128 partitions via a K=1 ones-matmul into PSUM.
            idx1 = cpool.tile([1, RK], BF16)
            nc.scalar.dma_start(out=idx1, in_=idxb[0:1, :])
            pe_sb = cpool.tile([P, 1 + D], F32)
            nc.sync.dma_start(out=pe_sb, in_=pe[:, :])
            pcol = pe_sb[:, 0:1]
            ones = cpool.tile([1, P], BF16)
            nc.vector.memset(ones, 1.0)

            # warm the ScalarE activation table during the input DMAs
            warm = cpool.tile([P, 1], F32)
            nc.scalar.copy(out=warm, in_=pcol)

            # countsT[n, r] = sum_k (idx[r,k] == n), per r-half, straight
            # from the PSUM broadcast tiles (is_equal is exact on f32 ints)
            counts = cpool.tile([P, R], BF16)
            ck = cpool.tile([P, R], BF16)

            def emit_hist(h, ppool, ones_t, pcol_t):
                pts = []
                RQ = RH // 2  # 400
                for k in range(K):
                    # 2 full PSUM banks; each matmult writes one bank only
                    # (<=512 fp32 cols, and start=True resets whole banks)
                    pt = ppool.tile([P, 1024], F32, tag="pidx")
                    c0 = k * R + h * RH
                    for q in range(2):
                        nc.tensor.matmul(
                            out=pt[:, q * 512 : q * 512 + RQ],
                            lhsT=ones_t[:, :],
                            rhs=idx1[:, c0 + q * RQ : c0 + (q + 1) * RQ],
                            start=True, stop=True,
                        )
                    pts.append(pt)
                cs = counts[:, h * RH : (h + 1) * RH]
                ks = ck[:, h * RH : (h + 1) * RH]

                def ts_eq(dst, k):
                    # plain per-bank slices only (no strided PSUM views)
                    for q in range(2):
                        nc.vector.tensor_scalar(
                            out=dst[:, q * RQ : (q + 1) * RQ],
                            in0=pts[k][:, q * 512 : q * 512 + RQ],
                            scalar1=pcol_t, scalar2=None, op0=is_eq,
                        )

                ts_eq(cs, 0)
                ts_eq(ks, 1)
                nc.vector.tensor_tensor(out=cs, in0=cs, in1=ks, op=add)
                ts_eq(ks, 2)
                nc.vector.tensor_tensor(out=cs, in0=cs, in1=ks, op=add)
                ts_eq(ks, 3)
                nc.vector.tensor_tensor(out=cs, in0=cs, in1=ks, op=add)

            out3 = out.rearrange("p (d r) -> p d r", r=R)

            def emit_halfgroup(d0, d1, h):
                gd = d1 - d0
                ob = hpool.tile([P, gd * RH], BF16, tag="hob")
                for i in range(gd):
                    nc.vector.tensor_scalar(
                        out=ob[:, i * RH : (i + 1) * RH],
                        in0=counts[:, h * RH : (h + 1) * RH],
                        scalar1=pe_sb[:, 1 + (d0 + i) : 2 + (d0 + i)],
                        scalar2=None, op0=mult,
                    )
                nc.sync.dma_start(
                    out=out3[:, d0:d1, h * RH : (h + 1) * RH],
                    in_=ob.rearrange("p (d r) -> p d r", r=RH),
                )
                return ob

            def emit_vgroup(d0, d1):
                ob = vpool.tile([P, (d1 - d0) * R], BF16, tag="vob")
                for i in range(d1 - d0):
                    nc.vector.tensor_scalar(
                        out=ob[:, i * R : (i + 1) * R], in0=counts,
                        scalar1=pe_sb[:, 1 + (d0 + i) : 2 + (d0 + i)],
                        scalar2=None, op0=mult,
                    )
                nc.sync.dma_start(out=out[:, d0 * R : d1 * R], in_=ob)

            def emit_sgroup(d0, d1):
                ob = spool.tile([P, (d1 - d0) * R], BF16, tag="sob")
                for i in range(d1 - d0):
                    nc.scalar.mul(
                        out=ob[:, i * R : (i + 1) * R], in_=counts,
                        mul=pe_sb[:, 1 + (d0 + i) : 2 + (d0 + i)],
                    )
                nc.scalar.dma_start(out=out[:, d0 * R : d1 * R], in_=ob)

            bp = mybir.AluOpType.bypass
            with tc.psum_pool(name="pidx", bufs=4) as ppool:
                emit_hist(0, ppool, ones, pcol)
                last_ob = None
                for d0, d1 in _HALF_D:
                    last_ob = emit_halfgroup(d0, d1, 0)
                # RAW-only gates (bypass copies that read a gating region):
                # chain B's matmuls recycle chain A's PSUM slots, so their
                # ones operand is cloned with a read of chain A's final
                # compare output; chain B's DVE compares would otherwise be
                # hoisted ahead of the first half-group muls by the
                # scheduler, so their pcol operand is cloned with a read of
                # the last h0 tile.
                ones_b = cpool.tile([1, P], BF16)
                nc.vector.scalar_tensor_tensor(
                    out=ones_b, in0=ones, scalar=0.0, in1=ck[0:1, 0:P],
                    op0=bp, op1=bp,
                )
                pcol_b = cpool.tile([P, 1], F32)
                nc.vector.scalar_tensor_tensor(
                    out=pcol_b, in0=pcol, scalar=0.0, in1=last_ob[:, 0:1],
                    op0=bp, op1=bp,
                )
                emit_hist(1, ppool, ones_b, pcol_b)
            for d0, d1 in _SCE_D:
                emit_sgroup(d0, d1)
            for d0, d1 in _HALF_D:
                emit_halfgroup(d0, d1, 1)
            for d0, d1 in _DVE_D:
                emit_vgroup(d0, d1)

    nc.finalize()
    return nc


def _get_nc():
    if "nc" not in _NC_CACHE:
        _NC_CACHE["nc"] = _build_nc()
    return _NC_CACHE["nc"]


def _prepare_in_maps(concepts, emb_table):
    concepts = np.asarray(concepts)
    emb = np.asarray(emb_table, dtype=np.float32)

    # per-core k-major index rows (single partition; device broadcasts)
    conc = concepts.reshape(NCORES, R, K)
    idx_dev = np.ascontiguousarray(
        conc.transpose(0, 2, 1).reshape(NCORES, 1, RK).astype(ml_dtypes.bfloat16)
    )

    pe = np.empty((P, 1 + D), dtype=np.float32)
    pe[:, 0] = np.arange(P, dtype=np.float32)
    pe[:, 1:] = emb

    return [{"idxb": idx_dev[i], "pe": pe} for i in range(NCORES)]


def _run(concepts, emb_table, **spmd_kwargs):
    nc = _get_nc()
    in_maps = _prepare_in_maps(concepts, emb_table)
    res = run_bass_kernel_spmd(nc, in_maps, core_ids=list(range(NCORES)), **spmd_kwargs)
    # shards are [128(n), 64(d)*1600(r)] bf16; -> [r, n, d], upcast exactly
    u16 = np.stack(
        [np.asarray(res.results[i]["out"]).view(np.uint16) for i in range(NCORES)]
    ).reshape(NCORES, N, D, R)
    u16 = u16.transpose(0, 3, 1, 2)  # -> [core, r, n, d]
    f32 = (u16.astype(np.uint32) << 16).view(np.float32)
    out = f32.reshape(B, S, N, D)
    return out, res


def kernel(concepts, emb_table):
    out, _ = _run(concepts, emb_table)
    return out
